# revision 62
# baseline (speedup 1.0000x reference)
"""Bahdanau additive attention kernel for 8 Trainium2 NeuronCores.

Data-parallel over batch: B=64 -> 8 batches per core. No collectives.

Per-batch math (reference):
  Wa   = dec @ Wa_w.T + Wa_b                       [1, H]
  Ua   = enc @ Ua_w.T + Ua_b                       [Te, H]
  s    = tanh(Ua + Wa) @ Va_w.T  (+ Va_b, dropped: softmax shift-invariant)
  w    = softmax(s)                                 [Te]
  ctx  = w @ enc                                    [1, De]

Layout: the big Ua matmul runs in the H-on-partitions orientation
(out[h, t]) with the Ua weights stationary and the fp8 encoder tiles
moving (perf_mode=DoubleRow).  That makes the Wa+bias term a
per-partition scalar, so it fuses into the tanh on ScalarE
(out = tanh(psum/1024 + bias)), and Va becomes the moving operand of
K=128 N=1 score-reduction matmuls (scores accumulate directly in score
column layout [128t', tci]).  exp runs on a [128, 8] tile; the context
and the softmax denominator (replicated to all partitions by an
all-ones stationary matmul) are again N=1 matmuls sharing one PSUM
accumulation group, with the final 1/sum scale on the otherwise-idle
DVE.

fp8 scaling: enc is quantized as enc*16 and the Ua/Wa weights as w*64
(all powers of two, undone exactly by the tanh activation's scale
argument 1/1024).  The scaling moves the small-magnitude weights out of
the fp8e4m3 subnormal range.  Measured HW rel err 1.32e-2 (gate 2e-2,
fixed-seed inputs so deterministic).

Cost-model timeline 82.5us/core (baseline 282.8us): ScalarE is the
critical resource (64 tanh ops over [128,1024] = 68us busy, ~98%
packed from 7.8us to 78.7us); TensorE ~56us and the serialized DMA
model ~77us (25MB/core at 360GB/s) run just underneath; ~3.8us
softmax->store tail (mostly fixed DMA gen/DGE/semaphore latency).
Startup: DMA order (WaPB weight half -> dec-fused Ua weight half ->
enc t-halves -> remaining halves), a split first tile, a t~0 dummy
activation that hoists the 1.3us activation-table load, and ~55
all-ones warm-up matmuls that complete the PE pstate ramp before the
first real matmul group.
Non-obvious pitfalls hit on real HW: gpsimd.partition_all_reduce
crashes the exec unit (NRT status 101) - stick to partition_broadcast /
matmul patterns; fp32 lhsT matmul and scalar-engine HWDGE stores are
fine.
"""

import os
import sys

import numpy as np
import ml_dtypes

for _p in ("/opt/trn_rl_repo",):
    if _p not in sys.path and os.path.isdir(_p):
        sys.path.append(_p)

import concourse.tile as tile
import concourse.mybir as mybir
from concourse import bacc
from concourse.bass import ts
from concourse.bass_utils import run_bass_kernel_spmd

B, T, D, H = 64, 1024, 1024, 1024
NCORES = 8
BPC = B // NCORES  # batches per core
P = 128
DC = D // P  # 8 contraction chunks
TC = T // P  # 8 t chunks
HT = H // P  # 8 h chunks

ENC_SCALE = 16.0  # fp8 quantization scale for encoder/decoder activations
W_SCALE = 64.0  # fp8 quantization scale for Ua_w / Wa_w
TOT_SCALE = ENC_SCALE * W_SCALE  # combined scale on the matmul PSUM

BF = mybir.dt.bfloat16
F8 = mybir.dt.float8e4
F32 = mybir.dt.float32
AF = mybir.ActivationFunctionType
ALU = mybir.AluOpType
DR = mybir.MatmulPerfMode.DoubleRow


def build_bass(
    eb_bufs: int = 4,
    nb_bufs: int = 4,
    pu_bufs: int = 3,
    th_bufs: int = 14,
    n_batches: int = BPC,
    mid_ht: int = 2,
    warm_a: int = 40,
    warm_b: int = 15,
    sc_bufs: int = 2,
):
    nc = bacc.Bacc("TRN2", target_bir_lowering=False, debug=False)

    # Ua weights with the decoder columns prepended (cols 0:8 = dec*16) and
    # padded to 1040 so the DoubleRow pair-dim stride stays 16B-aligned.
    # Fusing dec here avoids an 8-byte-row DMA paying per-descriptor minimums.
    HU = 8 + H + 8  # dec | uaw | pad
    encT = nc.dram_tensor("encT", [BPC, D, T], F8, kind="ExternalInput")
    encN = nc.dram_tensor("encN", [BPC, T, D], BF, kind="ExternalInput")
    uawX = nc.dram_tensor("uawX", [D, HU], F8, kind="ExternalInput")
    wawT = nc.dram_tensor("wawT", [D, H], F8, kind="ExternalInput")
    bsum = nc.dram_tensor("bsum", [1, H], BF, kind="ExternalInput")  # x1024
    vac = nc.dram_tensor("vac", [P, HT], BF, kind="ExternalInput")
    out = nc.dram_tensor("out", [BPC, P, DC], F32, kind="ExternalOutput")

    with tile.TileContext(nc) as tc:
        with (
            tc.tile_pool(name="const", bufs=1) as cpool,
            tc.tile_pool(name="enc", bufs=2) as epool,
            tc.tile_pool(name="work", bufs=2) as wpool,
            tc.tile_pool(name="pu", bufs=pu_bufs, space="PSUM") as pupool,
            tc.tile_pool(name="pc", bufs=2, space="PSUM") as pcpool,
        ):
            # --- resident weights / constants (DMA order matters: the
            # Wa weights come first so the WaPB bias columns are ready
            # when the first tanh fires) ---
            WW = cpool.tile([P, DC, H], F8, tag="WW")
            ww_src = wawT.ap().rearrange("(dc p) h -> p dc h", p=P)
            nc.sync.dma_start(WW[:, :, 0:512], ww_src[:, :, 0:512])
            BS = cpool.tile([1, H], BF, tag="BS")
            nc.sync.dma_start(BS[:], bsum.ap())
            UW = cpool.tile([P, DC, HU], F8, tag="UW")
            uw_src = uawX.ap().rearrange("(dc p) h -> p dc h", p=P)
            nc.sync.dma_start(UW[:, :, 0:520], uw_src[:, :, 0:520])

            ONE8 = cpool.tile([1, BPC], BF, tag="ONE8")
            nc.vector.memset(ONE8[:], 1.0)
            ONEPP = cpool.tile([P, P], BF, tag="ONEPP")
            nc.vector.memset(ONEPP[:], 1.0)

            # dummy 1-element tanh: pulls the auto-inserted 1.3us activation
            # table load to t~0 (it otherwise lands right before the first
            # real tanh, delaying the whole ScalarE stream)
            DUMT = cpool.tile([1, 1], BF, tag="DUMT")
            nc.scalar.activation(DUMT[:], ONE8[:, 0:1], AF.Tanh)

            def enc_dma_t(b):
                EB = epool.tile([P, DC, T], F8, tag="EB", bufs=eb_bufs)
                src = encT.ap()[b].rearrange("(dc p) t -> p dc t", p=P)
                if b == 0:
                    # batch 0 startup: weights arrive in h-column halves
                    # (h-tiles 0-3 need only the first), enc in t-halves —
                    # the first tanh fires ~3us before the weights finish
                    nc.sync.dma_start(EB[:, :, 0:512], src[:, :, 0:512])
                    nc.sync.dma_start(EB[:, :, 512:1024], src[:, :, 512:1024])
                    nc.sync.dma_start(WW[:, :, 512:1024], ww_src[:, :, 512:1024])
                    nc.sync.dma_start(UW[:, :, 520:HU], uw_src[:, :, 520:HU])
                else:
                    nc.sync.dma_start(EB[:], src)
                return EB

            # VAC is only needed by the first score stage; keep it off the
            # startup critical path (WW/UW/EB0 gate the first tanh)
            VAC = cpool.tile([P, HT], BF, tag="VAC")

            def enc_dma_n(b):
                NB = epool.tile([P, TC, D], BF, tag="NB", bufs=nb_bufs)
                nc.sync.dma_start(NB[:], encN.ap()[b].rearrange("(tc p) d -> p tc d", p=P))
                return NB

            # --- WaPB[h, b] = (dec*16) @ (Wa_w*64).T + (Wa_b+Ua_b)*1024,
            # in h-column layout, descaled to fp32 once.  Emitted in halves:
            # h-tiles 4-7 wait for WW's second DMA chunk, so they are spliced
            # into batch 0's Ua stage to keep the PE FIFO unblocked ---
            WaPBc = cpool.tile([P, HT, BPC], F32, tag="WaPBc")

            def prologue(hts):
                for ht in hts:
                    PW = pcpool.tile([P, BPC], F32, tag="sc8", bufs=sc_bufs, name=f"pw{ht}")
                    for dc in range(DC):
                        nc.tensor.matmul(
                            PW[:],
                            WW[:, dc, ts(ht, P)],
                            UW[:, dc, 0:BPC],
                            start=(dc == 0),
                            stop=False,
                        )
                    nc.tensor.matmul(
                        PW[:], BS[:, ts(ht, P)], ONE8[:], start=False, stop=True
                    )
                    nc.vector.tensor_scalar_mul(WaPBc[:, ht, :], PW[:], 1.0 / TOT_SCALE)

            # PE warm-up: garbage all-ones matmuls keep the PE continuously
            # busy from t~0.5us so the cost model's 3us pstate ramp finishes
            # before the first real matmul group (which would otherwise run
            # at the cold 1.2GHz-equivalent rate).  Split around the WaPB
            # prologue so the bias columns aren't pushed late.
            if warm_a or warm_b:
                DPU = pupool.tile([P, T], F32, tag="pu", name="dpu")
                for i in range(warm_a):
                    nc.tensor.matmul(
                        DPU[:, 0:P], ONEPP[:], ONEPP[:],
                        start=(i == 0), stop=False,
                    )
            prologue(range(0, HT // 2))
            if warm_a or warm_b:
                for i in range(warm_b):
                    nc.tensor.matmul(
                        DPU[:, 0:P], ONEPP[:], ONEPP[:],
                        start=False, stop=(i == warm_b - 1),
                    )
                if warm_b == 0:
                    nc.tensor.matmul(
                        DPU[:, 0:P], ONEPP[:], ONEPP[:], start=False, stop=True
                    )

            def ua_stage(b, EB, mid=None):
                """64 DoubleRow matmuls -> 8 fused bias+tanh tiles [128h, 1024t].

                ``mid`` (the previous batch's score/ctx emission) is spliced in
                after a couple of h-tiles so its exp lands early in the
                strict-FIFO Activation queue without gating this batch's first
                tanh."""
                THs = []
                for ht in range(HT):
                    split0 = b == 0 and ht == 0
                    if split0:
                        # separate per-half PSUM tiles: PSUM deps are
                        # whole-tile, so one [P, T] tile would make the first
                        # half-tanh wait for the second half's matmul group
                        PUh = [
                            pupool.tile([P, 512], F32, tag="pu", name=f"pu_{b}_{ht}_{th}")
                            for th in range(2)
                        ]
                    else:
                        PU = pupool.tile([P, T], F32, tag="pu", name=f"pu_{b}_{ht}")
                    for th in range(2):
                        dst = PUh[th][:] if split0 else PU[:, ts(th, 512)]
                        for g in range(DC // 2):
                            nc.tensor.matmul(
                                dst,
                                UW[:, 2 * g : 2 * g + 2, 8 + ht * P : 8 + (ht + 1) * P],
                                EB[:, 2 * g : 2 * g + 2, ts(th, 512)],
                                start=(g == 0),
                                stop=(g == DC // 2 - 1),
                                perf_mode=DR,
                            )
                    if b == 0 and ht == 2:
                        prologue(range(HT // 2, HT))
                    TH = wpool.tile([P, T], BF, tag="TH", bufs=th_bufs, name=f"th_{b}_{ht}")
                    act_out = TH
                    if split0:
                        # batch 0's first tile activates in t-halves so the
                        # ScalarE stream starts before EB0 fully lands
                        for th in range(2):
                            nc.scalar.activation(
                                act_out[:, ts(th, 512)],
                                PUh[th][:],
                                AF.Tanh,
                                bias=WaPBc[:, ht, b : b + 1],
                                scale=1.0 / TOT_SCALE,
                            )
                    else:
                        nc.scalar.activation(
                            act_out[:],
                            PU[:],
                            AF.Tanh,
                            bias=WaPBc[:, ht, b : b + 1],
                            scale=1.0 / TOT_SCALE,
                        )
                    THs.append(TH)
                    if ht == mid_ht - 1 and mid is not None:
                        mid()
                if mid is not None and mid_ht > HT:
                    mid()
                return THs

            def score_ctx_stage(b, THs, NB):
                # scores in column layout [128t', tci]: one accumulation
                # group, 64 K=128/N=1 matmuls with the Va column moving
                SCp = pcpool.tile([P, TC], F32, tag="sc8", bufs=sc_bufs, name=f"sc{b}")
                # ht-outer: the matmuls gated by the last tanh tile are the
                # final 8 of the group, so exp fires right after it lands
                for ht in range(HT):
                    for tci in range(TC):
                        nc.tensor.matmul(
                            SCp[:, tci : tci + 1],
                            THs[ht][:, ts(tci, P)],
                            VAC[:, ht : ht + 1],
                            start=(tci == 0 and ht == 0),
                            stop=(tci == TC - 1 and ht == HT - 1),
                        )
                EW = wpool.tile([P, TC], BF, tag="EW")
                nc.scalar.activation(EW[:], SCp[:], AF.Exp)
                # ctx reuses the sc8 PSUM ring: SCp dies at the exp, so the
                # ring alternates SC_b / CTX_b with bufs=2.  One accumulation
                # group: the context lands in columns 0-7 and sum(exp) in
                # column 8, replicated to every partition by all-ones
                # stationary matmuls (no broadcast hop).  For the last batch
                # the sum matmuls go first so the reciprocal -> scale -> store
                # tail fires as early as possible.
                CTXp = pcpool.tile([P, 12], F32, tag="sc8", bufs=sc_bufs, name=f"ctx{b}")
                ops = [
                    (CTXp[:, 8:9], ONEPP[:], EW[:, tci : tci + 1])
                    for tci in range(TC)
                ]
                ctx_ops = [
                    (CTXp[:, dc : dc + 1], NB[:, tci, ts(dc, P)], EW[:, tci : tci + 1])
                    for dc in range(DC)
                    for tci in range(TC)
                ]
                if b == n_batches - 1:
                    ops = ops + ctx_ops
                else:
                    ops = ctx_ops + ops
                for i, (o, l, r) in enumerate(ops):
                    nc.tensor.matmul(
                        o, l, r, start=(i == 0), stop=(i == len(ops) - 1)
                    )
                INV128 = wpool.tile([P, 1], F32, tag="INV128")
                nc.vector.reciprocal(INV128[:], CTXp[:, 8:9])
                OUTt = wpool.tile([P, DC], F32, tag="OUTt", bufs=4)
                nc.vector.tensor_scalar_mul(OUTt[:], CTXp[:, 0:DC], INV128[:])
                # mid-stream stores ride the software DGE so they never
                # head-of-line block the input loads on the sync queue; the
                # last one takes the by-then-empty SP HWDGE (lowest
                # gen+DGE-delay of all queues)
                if b == n_batches - 1:
                    nc.sync.dma_start(out.ap()[b], OUTt[:])
                else:
                    nc.gpsimd.dma_start(out.ap()[b], OUTt[:])

            # --- software pipeline: EB loads run one batch ahead; the
            # score/ctx stage of batch b-1 is spliced into the Ua stage of
            # b (after mid_ht tiles) so its exp lands early in the
            # strict-FIFO Activation queue ---
            EBs = {0: enc_dma_t(0)}
            nc.sync.dma_start(VAC[:], vac.ap())
            prev = None
            for b in range(n_batches):
                if b + 1 < n_batches:
                    EBs[b + 1] = enc_dma_t(b + 1)
                NB = enc_dma_n(b)
                mid = None
                if prev is not None:
                    pb, pTHs, pNB = b - 1, prev[0], prev[1]
                    mid = lambda: score_ctx_stage(pb, pTHs, pNB)
                THs = ua_stage(b, EBs.pop(b), mid=mid)
                prev = (THs, NB)
            score_ctx_stage(n_batches - 1, *prev)

    nc.finalize()
    return nc


_NC = None


def _get_nc():
    global _NC
    if _NC is None:
        _NC = build_bass()
    return _NC


LAST_RESULTS = None


def prepare_in_maps(inputs) -> list:
    enc = np.asarray(inputs["encoder_outputs"], dtype=np.float32)  # [B, T, D]
    dec = np.asarray(inputs["decoder_outputs"], dtype=np.float32)[:, 0, :]  # [B, D]
    Wa_w = np.asarray(inputs["Wa_w"], dtype=np.float32)
    Wa_b = np.asarray(inputs["Wa_b"], dtype=np.float32)
    Ua_w = np.asarray(inputs["Ua_w"], dtype=np.float32)
    Ua_b = np.asarray(inputs["Ua_b"], dtype=np.float32)
    Va_w = np.asarray(inputs["Va_w"], dtype=np.float32)
    # Va_b dropped: softmax(s + c) == softmax(s)

    bf16 = ml_dtypes.bfloat16
    f8 = ml_dtypes.float8_e4m3

    encN_all = enc.astype(bf16).reshape(NCORES, BPC, T, D)
    encT_all = (
        (np.ascontiguousarray(enc.transpose(0, 2, 1)) * ENC_SCALE)
        .astype(f8)
        .reshape(NCORES, BPC, D, T)
    )
    decT_all = (
        np.ascontiguousarray(dec.reshape(NCORES, BPC, D).transpose(0, 2, 1)) * ENC_SCALE
    ).astype(f8)  # [NCORES, D, BPC]
    uawT8 = (np.ascontiguousarray(Ua_w.T) * W_SCALE).astype(f8)
    wawT8 = (np.ascontiguousarray(Wa_w.T) * W_SCALE).astype(f8)
    pad8 = np.zeros((D, 8), dtype=f8)
    uawX_all = [
        np.ascontiguousarray(np.concatenate([decT_all[c], uawT8, pad8], axis=1))
        for c in range(NCORES)
    ]
    bsumS = ((Wa_b + Ua_b) * TOT_SCALE).reshape(1, H).astype(bf16)
    vac = np.ascontiguousarray(Va_w.reshape(HT, P).T).astype(bf16)  # [P, HT]

    return [
        {
            "encT": np.ascontiguousarray(encT_all[c]),
            "encN": np.ascontiguousarray(encN_all[c]),
            "uawX": uawX_all[c],
            "wawT": wawT8,
            "bsum": bsumS,
            "vac": vac,
        }
        for c in range(NCORES)
    ]


def kernel(**inputs) -> np.ndarray:
    in_maps = prepare_in_maps(inputs)
    nc = _get_nc()
    trace = bool(int(os.environ.get("KERNEL_TRACE", "0")))
    try:
        res = run_bass_kernel_spmd(
            nc, in_maps, core_ids=list(range(NCORES)), trace=trace
        )
    except ModuleNotFoundError:
        # axon clients without the NTFF hook (antenv.axon_hooks) cannot trace;
        # retry untraced rather than failing the whole run
        os.environ["BASS_NEVER_TRACE"] = "1"
        res = run_bass_kernel_spmd(
            nc, in_maps, core_ids=list(range(NCORES)), trace=False
        )
    global LAST_RESULTS
    LAST_RESULTS = res

    # out[b, p, dc] holds ctx[b, dc*128 + p]
    outs = [
        res.results[c]["out"].transpose(0, 2, 1).reshape(BPC, D)
        for c in range(NCORES)
    ]
    return np.concatenate(outs, axis=0).reshape(B, 1, D).astype(np.float32)
